# revision 10
# baseline (speedup 1.0000x reference)
"""LookAheadMask kernel for Trainium2 — in-place, merged diag writes, v4.

out[b, r, c] = 1.0 if c > r else x[b, r, c], for x of shape (8, 4096, 4096) f32.

Sharding: batch dim across 8 NeuronCores (data parallel, no communication).
The output aliases the input buffer (lowering_input_output_aliases={0: 0}),
so the strictly-lower triangle never moves: ~4 MiB read + ~34 MiB written
per core.

Measured head model (v1-v3 traces): SP ring ~8.3 ns/descriptor and ~440+
GB/s on big descriptors; ACT ring ~440 GB/s for >=4 KiB descriptors but
~19 ns/desc below ~2 KiB; framework preamble ~9 us.  Plan:

  - Diag gather (4096 x 1 KiB descs, desc-bound, unavoidable) runs on SP,
    in 4 chunks so the 4 affine_selects pipeline behind it.
  - No scatter: diag blocks leave SBUF as the leading 128 cols of
    [128 x 1024] rectangles sourced from diag_sel, a [128, 32*1024] tile
    pre-memset to 1.0 in quarters (2 on DVE, 2 on gpsimd) into which
    gpsimd affine_selects only the diagonal 128-col windows.
  - ACT gets the wide pure-ones rectangles (>=4.5 KiB descs) and the
    merged c0 chunk (blocks 0-15); SP gets the narrow pure rects, merged
    c1 (blocks 16-23) and the clipped blocks 24-31; six mid-width pure
    rects go to the gpsimd SWDGE queue (tracked by wsem to measure it).
"""

import numpy as np

S = 4096
P = 128
NB = S // P  # 32
N_CORES = 8
W = 256  # diag gather window width (1 KiB descriptors)
MW = 1024  # merged diag-rectangle width (4 KiB descriptors)
DB = P * S + P  # element stride between consecutive diagonal blocks

SWDGE_BLOCKS = [12, 15, 16, 17, 18, 19]  # pure-ones rects on the SWDGE queue
SP_BLOCKS = [0, 20, 21, 22, 23]  # pure-ones rects on the SP ring
ACT_BLOCKS = [i for i in range(24) if i not in SWDGE_BLOCKS + SP_BLOCKS]

_cached = None


def _build():
    from concourse import bass, mybir

    nc = bass.Bass(target_bir_lowering=True, enable_partition_id=False)
    x = nc.dram_tensor("x", [S, S], mybir.dt.float32, kind="ExternalInput")
    out = nc.dram_tensor("out", [S, S], mybir.dt.float32, kind="ExternalOutput")

    N_WRITES = 18 + 2 + 8  # ring pure ones + merged chunks + clipped (SWDGE rects on wsem)

    def pure_ones(eng, blocks, ones, sem):
        for i in blocks:
            r0 = i * P
            w = S - r0 - MW
            eng.dma_start(
                out=out[r0 : r0 + P, r0 + MW : S], in_=ones[:, :w]
            ).then_inc(sem, 16)

    def gather_chunk(eng, b0, nblk, gsa):
        eng.dma_start(
            out=bass.AP(diag_in2_h[0], b0 * W + W, [[NB * W, P], [W, nblk], [1, W]]),
            in_=bass.AP(
                x, (b0 + 1) * DB + P - W, [[S, P], [DB, nblk], [1, W]]
            ),
        ).then_inc(gsa, 16)

    diag_in2_h = [None]

    with (
        nc.Block() as block,
        nc.semaphore("dsem") as dsem,  # all output-write DMA completions
        nc.semaphore("gsa") as gsa,  # gather chunks (SP ring)
        nc.semaphore("msem") as msem,  # ones memset done
        nc.semaphore("m2") as m2,  # diag_sel DVE-quarter memsets done
        nc.semaphore("asem") as asem,  # affine_selects done
        nc.semaphore("wsem") as wsem,  # SWDGE pure-ones done (timing probe)
        nc.sbuf_tensor("ones", [P, S - MW], mybir.dt.float32) as ones,
        nc.sbuf_tensor("diag_in2", [P, NB * W], mybir.dt.float32) as diag_in2,
        nc.sbuf_tensor("diag_sel", [P, NB * MW], mybir.dt.float32) as diag_sel,
    ):
        diag_in2_h[0] = diag_in2

        @block.vector
        def _(vector: bass.BassVectorEngine):
            vector.memset(ones[:, :], 1.0).then_inc(msem, 1)
            vector.memset(diag_sel[:, : 8 * MW], 1.0).then_inc(m2, 1)
            vector.memset(diag_sel[:, 8 * MW : 16 * MW], 1.0).then_inc(m2, 1)

        @block.sync
        def _(sync: bass.BassEngine):
            # Diag gather on the fast-descriptor SP head, 4 chunks of 8
            # blocks. Block 0's window would start before the tensor, so it
            # gets its own 128-col load (chunk 0 covers blocks 1-7).
            sync.dma_start(
                out=bass.AP(diag_in2, W - P, [[NB * W, P], [1, P]]),
                in_=x[0:P, 0:P],
            ).then_inc(gsa, 16)
            gather_chunk(sync, 0, 7, gsa)
            gather_chunk(sync, 7, 8, gsa)
            gather_chunk(sync, 15, 8, gsa)
            gather_chunk(sync, 23, 8, gsa)
            sync.wait_ge(msem, 1)
            pure_ones(sync, SP_BLOCKS, ones, dsem)
            sync.wait_ge(asem, 3)
            # Merged rectangles for diag blocks 16-23.
            sync.dma_start(
                out=bass.AP(out, 16 * DB, [[S, P], [DB, 8], [1, MW]]),
                in_=bass.AP(diag_sel, 16 * MW, [[NB * MW, P], [MW, 8], [1, MW]]),
            ).then_inc(dsem, 16)
            sync.wait_ge(asem, 4)
            # Blocks 24-31: clipped merged rect covers the whole row span.
            for b in range(24, 32):
                r0 = b * P
                w = S - r0
                sync.dma_start(
                    out=out[r0 : r0 + P, r0:S],
                    in_=bass.AP(diag_sel, b * MW, [[NB * MW, P], [1, w]]),
                ).then_inc(dsem, 16)
            sync.wait_ge(dsem, 16 * N_WRITES)
            sync.wait_ge(wsem, 16 * len(SWDGE_BLOCKS))

        @block.scalar
        def _(scalar: bass.BassEngine):
            scalar.wait_ge(msem, 1)
            pure_ones(scalar, ACT_BLOCKS, ones, dsem)
            scalar.wait_ge(asem, 2)
            # Merged rectangles for diag blocks 0-15 (4 KiB descriptors —
            # fine on ACT; only <=2 KiB descriptors are slow there).
            scalar.dma_start(
                out=bass.AP(out, 0, [[S, P], [DB, 16], [1, MW]]),
                in_=bass.AP(diag_sel, 0, [[NB * MW, P], [MW, 16], [1, MW]]),
            ).then_inc(dsem, 16)

        @block.gpsimd
        def _(gpsimd: bass.BassGpSimd):
            gpsimd.memset(diag_sel[:, 16 * MW : 24 * MW], 1.0)
            gpsimd.memset(diag_sel[:, 24 * MW :], 1.0)

            def select(q):
                # iota[p, c] = p - c; keep x where p >= c (at/below diag),
                # else 1.0. Writes ONLY the 128 diag cols of each 1024-wide
                # window; the other 896 cols keep the memset 1.0.
                gpsimd.affine_select(
                    out=bass.AP(
                        diag_sel, q * 8 * MW, [[NB * MW, P], [MW, 8], [1, P]]
                    ),
                    in_=bass.AP(
                        diag_in2,
                        q * 8 * W + W - P,
                        [[NB * W, P], [W, 8], [1, P]],
                    ),
                    pattern=[[0, 8], [-1, P]],
                    base=0,
                    channel_multiplier=1,
                    compare_op=mybir.AluOpType.is_ge,
                    fill=1.0,
                ).then_inc(asem, 1)

            gpsimd.wait_ge(gsa, 32)  # block 0 + chunk 1-7
            gpsimd.wait_ge(m2, 1)
            select(0)
            gpsimd.wait_ge(msem, 1)
            pure_ones(gpsimd, SWDGE_BLOCKS[:2], ones, wsem)
            gpsimd.wait_ge(gsa, 48)
            gpsimd.wait_ge(m2, 2)
            select(1)
            pure_ones(gpsimd, SWDGE_BLOCKS[2:4], ones, wsem)
            gpsimd.wait_ge(gsa, 64)
            select(2)
            pure_ones(gpsimd, SWDGE_BLOCKS[4:], ones, wsem)
            gpsimd.wait_ge(gsa, 80)
            select(3)

    nc.finalize()
    return nc


def _make_runner():
    """Compile-once runner: jit(shard_map(_body)) over 8 cores with the
    output aliased to the (donated) input — mirrors
    bass2jax.run_bass_via_pjrt, plus lowering_input_output_aliases."""
    global _cached
    if _cached is not None:
        return _cached

    import jax
    from jax.sharding import Mesh, PartitionSpec
    from jax.experimental.shard_map import shard_map
    from concourse import bass2jax

    bass2jax.install_neuronx_cc_hook()
    nc = _build()

    def _body(xg):
        outs = bass2jax._bass_exec_p.bind(
            xg,
            out_avals=(jax.core.ShapedArray((S, S), np.float32),),
            in_names=("x",),
            out_names=("out",),
            lowering_input_output_aliases=((0, 0),),
            sim_require_finite=True,
            sim_require_nnan=True,
            nc=nc,
        )
        return tuple(outs)

    devices = jax.devices()[:N_CORES]
    assert len(devices) == N_CORES, f"need {N_CORES} devices, have {len(devices)}"
    mesh = Mesh(np.asarray(devices), ("core",))
    sharded = jax.jit(
        shard_map(
            _body,
            mesh=mesh,
            in_specs=(PartitionSpec("core"),),
            out_specs=(PartitionSpec("core"),),
            check_rep=False,
        ),
        donate_argnums=(0,),
        keep_unused=True,
    )
    _cached = (nc, sharded)
    return _cached


class _Result:
    def __init__(self, exec_time_ns=None, mean_exec_time_ns=None):
        self.exec_time_ns = exec_time_ns
        self.mean_exec_time_ns = mean_exec_time_ns


def _run(x_full: np.ndarray, trace: bool = False):
    nc, sharded = _make_runner()
    x_full = np.asarray(x_full, dtype=np.float32)
    xg = np.ascontiguousarray(x_full.reshape(N_CORES * S, S))

    if not trace:
        out = sharded(xg)[0]
        return np.asarray(out).reshape(N_CORES, S, S), _Result()

    # Trace path (test.py only): NTFF profile around the execution, then the
    # same gauge/perfetto pipeline run_bass_kernel_spmd uses under axon.
    import glob
    import os
    import tempfile

    from antenv.axon_hooks import get_axon_ntff_profile_hook
    from concourse import bass_utils as BU

    neff_dir = tempfile.mkdtemp()
    hook = get_axon_ntff_profile_hook()
    with hook(neff_dir, [0]):
        out = np.asarray(sharded(xg)[0])

    ntffs = glob.glob(os.path.join(neff_dir, "*_body*.ntff"))
    if not ntffs:
        return out.reshape(N_CORES, S, S), _Result()

    sharepath = BU.upload_artifacts(neff_dir)
    profile = BU.gauge.profiler.Profile(
        profile_path=BU.FishPath(neff_dir),
        kernel_dev_mode=True,
        profile_on_exit=False,
        bass_kernel=nc.m,
        offline_processing=True,
        fname="*_body*",
        annotate_hlo=False,
        metadata={"artifacts_path": sharepath},
    )
    perf = BU._process_ntff_profile(
        profile,
        neff_dir,
        nc,
        list(range(N_CORES)),
        None,
        False,
        {},
        trace_events=False,
    )
    return out.reshape(N_CORES, S, S), _Result(
        perf.exec_time_ns, perf.mean_exec_time_ns
    )


def kernel(x: np.ndarray) -> np.ndarray:
    out, _ = _run(x, trace=False)
    return out
